# revision 15
# baseline (speedup 1.0000x reference)
"""Trainium2 Bass kernel: autoregressive wavefunction log-prob (N=64, B=2048, H=512).

Sharding: step axis N across 8 cores, round-robin (core c owns global steps
c, c+8, ..., c+56).  Each core computes, for its 8 steps i:
    h1 = relu(X_i @ W1_i + b1_i)        X_i = prefix one-hots (masked into W1)
    h2 = relu(h1 @ W2_i + b2_i)
    d  = h2 @ (W3_i[:,0]-W3_i[:,1]) + (b3_i[0]-b3_i[1])
    logp_i = ln(sigmoid(sigma_i * d))   sigma = s0 - s1 in {+1,-1}
and returns a [128, 512] tile holding logp for (step j, batch-chunk b) at row
32*b + j; the host gathers and sums over steps/cores.

On-chip layout: activations are kept transposed [H, B] so W1 [2N,H] and
W2 [Hin,Hout] serve directly as matmul lhsT.  The ragged prefix (rows >= 2i
of the padded W1) is zeroed on the host so a single SPMD graph serves all
cores.  All matmuls run in bf16 with fp32 PSUM accumulation.  The scalar
collapse d uses an M=8 matmul whose lhsT has w3d in column j and zeros
elsewhere, accumulated across all steps into one PSUM bank via column-group
tiling (tile_position=(0,32b)).  ln(sigmoid(x)) is computed as -ln(1+exp(-x))
so Relu/Exp/Ln share one ACT table set (no mid-kernel table switches).
"""

import numpy as np
import ml_dtypes

import concourse.bass as bass
import concourse.mybir as mybir
import concourse.tile as tile
from concourse import bass_utils as _bu
from concourse.bass_utils import run_bass_kernel_spmd

N, B, H = 64, 2048, 512
NCORES = 8
NSTEP = N // NCORES          # 8 local steps per core
BCH = 512                    # batch chunk (one PSUM bank of fp32)
NB = B // BCH                # 4
NM = H // 128                # 4 h-chunks
K2N = 2 * N                  # 128, layer-1 contraction

BF = mybir.dt.bfloat16
F32 = mybir.dt.float32
NPBF = ml_dtypes.bfloat16

# set by test harness to capture profile/results
TRACE = False
LAST_RESULT = None

def _elide_redundant_ldweights(nc):
    """Tile lowers every matmul into an Ldweights+Matmult pair.  The loops
    below are ordered b-innermost so consecutive matmuls share the stationary
    operand; drop the repeated loads (the PE weight registers are untouched
    by intervening Matmults).  A dropped load's semaphore waits are kept as
    an EventSemaphore in the same PE stream."""
    n_elided = 0
    for fn in nc.m.functions:
        for blk in fn.blocks:
            new = []
            last_key = None
            for inst in blk.instructions:
                if isinstance(inst, mybir.InstMatmult):
                    new.append(inst)
                    continue
                if isinstance(inst, mybir.InstLdweights):
                    a = inst.ins[0]
                    key = (
                        a.memref, a.offset, str(a.ap), str(a.dtype),
                        str(inst.perf_mode), str(inst.tile_position),
                        str(inst.is_transpose),
                    )
                    if key == last_key:
                        si = inst.sync_info
                        if si is not None and (si.on_wait or si.on_update):
                            new.append(mybir.InstEventSemaphore(
                                name=f"{inst.name}-ldwelide",
                                engine=inst.engine,
                                sync_info=si,
                            ))
                        n_elided += 1
                        continue
                    last_key = key
                    new.append(inst)
                    continue
                if inst.engine == mybir.EngineType.PE:
                    last_key = None  # drains/branches etc: be conservative
                new.append(inst)
            blk.instructions = new
    return nc


def _thin_sem_incs(nc):
    """Every PE instruction carries a then_inc (~26ns serialized at the EVT_SEM
    register).  Only increments whose cumulative value is actually waited on
    are needed — PE executes its queue in order, so the v-th increment firing
    implies all prior work retired.  Drop unwaited increments and renumber the
    waits.  Applied only to semaphores updated exclusively by PE instructions
    with update_value 1 (never DMA/collective sems)."""
    blocks = [blk for fn in nc.m.functions for blk in fn.blocks]
    upd = {}    # sem_id -> list of (inst, cum_value)
    waited = {} # sem_id -> set of waited values
    ok = {}     # sem_id -> eligible
    for blk in blocks:
        for inst in blk.instructions:
            si = inst.sync_info
            if si is None:
                continue
            for w in (si.on_wait or []):
                if w.wait_mode == "sem-ge-imm" and w.wait_value is not None:
                    waited.setdefault(w.id, set()).add(w.wait_value)
                else:
                    ok[w.id] = False  # register waits etc: leave alone
            for u in (si.on_update or []):
                lst = upd.setdefault(u.id, [])
                lst.append((inst, (lst[-1][1] if lst else 0) + (u.update_value or 0)))
                is_pe = (
                    inst.engine == mybir.EngineType.PE
                    and u.update_value == 1
                    and getattr(u, "update_mode", "sem-inc") == "sem-inc"
                )
                if not is_pe:
                    ok[u.id] = False
    n_dropped = 0
    for sem_id, updates in upd.items():
        if not ok.get(sem_id, True):
            continue
        keep_vals = sorted(v for v in waited.get(sem_id, set()) if v > 0)
        remap = {}
        new_cum = 0
        ki = 0
        for inst, cum in updates:
            # keep this inc iff cum is the smallest update >= some waited value
            need = ki < len(keep_vals) and cum >= keep_vals[ki]
            if need:
                while ki < len(keep_vals) and keep_vals[ki] <= cum:
                    remap[keep_vals[ki]] = new_cum + 1
                    ki += 1
                new_cum += 1
            else:
                si = inst.sync_info
                nu = [u for u in (si.on_update or []) if u.id != sem_id]
                inst.sync_info = mybir.SyncInfo(
                    on_wait=list(si.on_wait or []), on_update=nu
                )
                n_dropped += 1
        for blk in blocks:
            for inst in blk.instructions:
                si = inst.sync_info
                if si is None or not si.on_wait:
                    continue
                changed = False
                nw = []
                for w in si.on_wait:
                    if w.id == sem_id and w.wait_value and w.wait_value > 0:
                        nw.append(mybir.SyncWait(
                            sync_type=w.sync_type, id=w.id,
                            ant_name=w.ant_name, wait_mode=w.wait_mode,
                            wait_value=remap[w.wait_value],
                        ))
                        changed = True
                    else:
                        nw.append(w)
                if changed:
                    inst.sync_info = mybir.SyncInfo(
                        on_wait=nw, on_update=list(si.on_update or [])
                    )
    return nc


def _legalize_waits(nc):
    """This walrus build encodes at most ONE semaphore wait per instruction
    (one NEURON_ISA_TPB_EVENTS slot).  Tile emits multi-wait sync_info; spill
    the extras onto standalone EventSemaphore instructions inserted just
    before, in the same engine's FIFO stream — semantically identical."""
    for fn in nc.m.functions:
        for blk in fn.blocks:
            new = []
            for inst in blk.instructions:
                si = inst.sync_info
                if si is not None and si.on_wait is not None and len(si.on_wait) > 1:
                    waits = list(si.on_wait)
                    for idx, w in enumerate(waits[:-1]):
                        new.append(mybir.InstEventSemaphore(
                            name=f"{inst.name}-spill{idx}",
                            engine=inst.engine,
                            sync_info=mybir.SyncInfo(on_wait=[w], on_update=[]),
                        ))
                    inst.sync_info = mybir.SyncInfo(
                        on_wait=[waits[-1]], on_update=list(si.on_update)
                    )
                new.append(inst)
            blk.instructions = new
    return nc


def build_graph():
    nc = bass.Bass()
    S_d = nc.declare_dram_parameter("S", [K2N, B], BF, False)
    W1_d = nc.declare_dram_parameter("W1", [NSTEP, K2N, H], BF, False)
    W2_d = nc.declare_dram_parameter("W2", [NSTEP, 128, NM * H], BF, False)
    B1_d = nc.declare_dram_parameter("B1", [128, NSTEP * NM], F32, False)
    B2_d = nc.declare_dram_parameter("B2", [128, NSTEP * NM], F32, False)
    W3D_d = nc.declare_dram_parameter("W3D", [128, NSTEP * NM * NSTEP], BF, False)
    B3D_d = nc.declare_dram_parameter("B3D", [128, 1], F32, False)
    SIG_d = nc.declare_dram_parameter("SIG", [128, BCH], F32, False)
    OUT_d = nc.declare_dram_parameter("out", [128, BCH], F32, True)

    add = mybir.AluOpType.add
    amax = mybir.AluOpType.max
    mult = mybir.AluOpType.mult
    Relu = mybir.ActivationFunctionType.Relu
    Exp = mybir.ActivationFunctionType.Exp
    Ln = mybir.ActivationFunctionType.Ln

    with tile.TileContext(nc) as tc:
        with (
            tc.tile_pool(name="const", bufs=1) as const,
            tc.tile_pool(name="w1p", bufs=2) as w1p,
            tc.tile_pool(name="w2p", bufs=2) as w2p,
            tc.tile_pool(name="h1p", bufs=18) as h1p,
            tc.tile_pool(name="h2p", bufs=20) as h2p,
            tc.tile_pool(name="tailp", bufs=2) as tailp,
            tc.tile_pool(name="pp", bufs=7, space="PSUM") as pp,
            tc.tile_pool(name="dp", bufs=1, space="PSUM") as dp,
        ):
            # ---- warmup: zeroed operands for PE HAM warmup + ACT table load
            wz = const.tile([128, BCH + 128], BF)
            nc.vector.memset(wz[:], 0.0)
            wact = const.tile([128, 1], F32)
            nc.vector.memset(wact[:], 0.0)
            # one table set covers Relu/Exp/Ln -> single ACT_TABLE_LOAD here
            nc.scalar.activation(wact[:], wact[:], Exp)
            nc.scalar.activation(wact[:], wact[:], Ln)
            nc.scalar.activation(wact[:], wact[:], Relu)

            # ---- startup DMAs, ordered by first use
            w1_first = w1p.tile([K2N, H], BF, tag="w1")
            nc.sync.dma_start(out=w1_first[:], in_=W1_d[0])
            S_sb = const.tile([K2N, B], BF)
            nc.sync.dma_start(out=S_sb[:], in_=S_d[:])
            B1_sb = const.tile([128, NSTEP * NM], F32)
            nc.sync.dma_start(out=B1_sb[:], in_=B1_d[:])
            w2_first = w2p.tile([128, NM * H], BF, tag="w2")
            nc.sync.dma_start(out=w2_first[:], in_=W2_d[0])
            B2_sb = const.tile([128, NSTEP * NM], F32)
            nc.sync.dma_start(out=B2_sb[:], in_=B2_d[:])
            W3D_sb = const.tile([128, NSTEP * NM * NSTEP], BF)
            nc.sync.dma_start(out=W3D_sb[:], in_=W3D_d[:])
            B3D_sb = const.tile([128, 1], F32)
            nc.sync.dma_start(out=B3D_sb[:], in_=B3D_d[:])
            SIG_sb = const.tile([128, BCH], F32)
            nc.sync.dma_start(out=SIG_sb[:], in_=SIG_d[:])

            # persistent d accumulator: row 32*b + j = (batch chunk b, step j)
            D1 = dp.tile([128, BCH], F32)
            nc.vector.memset(D1[:], 0.0)

            # PE warmup: harmless matmuls on zeroed SBUF while DMAs land
            wps = pp.tile([128, BCH], F32, tag="ps")
            for _ in range(14):
                nc.tensor.matmul(
                    wps[:], wz[:, BCH:BCH + 128], wz[:, 0:BCH],
                    start=True, stop=True, skip_group_check=True,
                )

            def emit_l3(j, k, h2):
                c0 = (j * NM + k) * NSTEP
                lhsT = W3D_sb[:, c0: c0 + NSTEP]
                for b in reversed(range(NB)):
                    nc.tensor.matmul(
                        D1[32 * b: 32 * b + NSTEP, :], lhsT, h2[(k, b)][:],
                        start=(j == 0 and k == 0),
                        stop=(j == NSTEP - 1 and k == NM - 1),
                        skip_group_check=True,
                        tile_position=(0, 32 * b),
                    )

            deferred_l3 = None  # (j, h2) — last k-chunk emitted after next L1
            for j in range(NSTEP):
                if j == 0:
                    w1, w2 = w1_first, w2_first
                else:
                    w1 = w1p.tile([K2N, H], BF, tag="w1")
                    nc.sync.dma_start(out=w1[:], in_=W1_d[j])
                    w2 = w2p.tile([128, NM * H], BF, tag="w2")
                    nc.sync.dma_start(out=w2[:], in_=W2_d[j])

                # ---- layer 1: h1T[m] = relu(W1[:,m]^T S + b1[m]), b-inner
                h1 = {}
                for m in range(NM):
                    bias = B1_sb[:, j * NM + m: j * NM + m + 1]
                    for b in range(NB):
                        bs = slice(b * BCH, (b + 1) * BCH)
                        ps = pp.tile([128, BCH], F32, tag="ps")
                        nc.tensor.matmul(
                            ps[:], w1[:, m * 128:(m + 1) * 128], S_sb[:, bs],
                            start=True, stop=True,
                        )
                        t = h1p.tile([128, BCH], BF, tag="h1")
                        if b % 2 == 0:
                            nc.scalar.activation(t[:], ps[:], Relu, bias=bias)
                        else:
                            nc.vector.tensor_scalar(
                                t[:], ps[:], bias, 0.0, op0=add, op1=amax,
                            )
                        h1[(m, b)] = t

                # previous step's deferred L3 k-chunk: its h2 epilogues are
                # long done, and the L1 matmuls above cover any residual lag
                if deferred_l3 is not None:
                    pj, ph2 = deferred_l3
                    emit_l3(pj, NM - 1, ph2)
                    deferred_l3 = None

                # ---- layer 2: k-accumulation descending, lhsT shared over b;
                # first matmul (k=3, b=3) waits on the newest h1 tile, which
                # covers every other h1 tick — no per-matmul waits after it
                h2 = {}
                for m in range(NM):
                    bias = B2_sb[:, j * NM + m: j * NM + m + 1]
                    pss = [
                        pp.tile([128, BCH], F32, tag="ps", name=f"ps2_{j}_{m}_{b}")
                        for b in range(NB)
                    ]
                    for k in reversed(range(NM)):
                        lhsT = w2[:, k * H + m * 128: k * H + (m + 1) * 128]
                        for b in reversed(range(NB)):
                            nc.tensor.matmul(
                                pss[b][:], lhsT, h1[(k, b)][:],
                                start=(k == NM - 1), stop=(k == 0),
                            )
                    for b in range(NB):
                        t = h2p.tile([128, BCH], BF, tag="h2")
                        if b % 2 == 1:
                            nc.scalar.activation(t[:], pss[b][:], Relu, bias=bias)
                        else:
                            nc.vector.tensor_scalar(
                                t[:], pss[b][:], bias, 0.0, op0=add, op1=amax,
                            )
                        h2[(m, b)] = t

                # ---- layer 3: D1[32b+j, :] += w3d_j^T @ h2[k][b] (col groups)
                for k in range(NM - 1):
                    emit_l3(j, k, h2)
                deferred_l3 = (j, h2)
            emit_l3(NSTEP - 1, NM - 1, deferred_l3[1])

            # ---- tail: logp = -ln(1 + exp(-sigma*(d + b3d))), one bank wide
            tt = tailp.tile([128, BCH], F32, tag="tt")
            nc.vector.scalar_tensor_tensor(
                tt[:], D1[:], B3D_sb[:, 0:1], SIG_sb[:], op0=add, op1=mult,
            )
            ex = tailp.tile([128, BCH], F32, tag="ex")
            nc.scalar.activation(ex[:], tt[:], Exp, scale=-1.0)
            e1 = tailp.tile([128, BCH], F32, tag="e1")
            nc.vector.tensor_scalar(e1[:], ex[:], 1.0, None, op0=add)
            lp = tailp.tile([128, BCH], F32, tag="lp")
            nc.scalar.activation(lp[:], e1[:], Ln)
            nc.sync.dma_start(out=OUT_d[:], in_=lp[:])

    return _legalize_waits(_thin_sem_incs(_elide_redundant_ldweights(nc)))


_NC_CACHE = None


def _get_graph():
    global _NC_CACHE
    if _NC_CACHE is None:
        _NC_CACHE = build_graph()
    return _NC_CACHE


def _prep_inputs(samples, W1, b1, W2, b2, W3, b3):
    samples = np.asarray(samples, np.float32)
    W1 = np.asarray(W1, np.float32)
    b1 = np.asarray(b1, np.float32)
    W2 = np.asarray(W2, np.float32)
    b2 = np.asarray(b2, np.float32)
    W3 = np.asarray(W3, np.float32)
    b3 = np.asarray(b3, np.float32)

    # S[2j+s, b] = samples[j, b, s]
    S = samples.transpose(0, 2, 1).reshape(K2N, B).astype(NPBF)
    # mask padded rows: row k of W1[i] is dead unless k < 2i
    row = np.arange(K2N)[None, :, None]
    step = np.arange(N)[:, None, None]
    W1m = np.where(row < 2 * step, W1, 0.0).astype(NPBF)
    w3d = (W3[:, :, 0] - W3[:, :, 1]).astype(np.float32)      # (N, H)
    b3d = (b3[:, 0] - b3[:, 1]).astype(np.float32)            # (N,)
    sig = (samples[:, :, 0] - samples[:, :, 1]).astype(np.float32)  # (N, B)

    in_maps = []
    for c in range(NCORES):
        steps = c + NCORES * np.arange(NSTEP)
        W2c = (
            W2[steps]
            .reshape(NSTEP, NM, 128, H)
            .transpose(0, 2, 1, 3)
            .reshape(NSTEP, 128, NM * H)
            .astype(NPBF)
        )
        B1c = (
            b1[steps].reshape(NSTEP, NM, 128).transpose(2, 0, 1)
            .reshape(128, NSTEP * NM).astype(np.float32)
        )
        B2c = (
            b2[steps].reshape(NSTEP, NM, 128).transpose(2, 0, 1)
            .reshape(128, NSTEP * NM).astype(np.float32)
        )
        # W3D[p, ((j*NM + k)*NSTEP) + jj] = w3d[steps[j], k*128+p] if jj == j
        W3Dc = np.zeros((128, NSTEP, NM, NSTEP), np.float32)
        for j in range(NSTEP):
            W3Dc[:, j, :, j] = w3d[steps[j]].reshape(NM, 128).T
        W3Dc = W3Dc.reshape(128, NSTEP * NM * NSTEP).astype(NPBF)

        # row 32*b + j layouts for the tail
        SIGc = np.zeros((128, BCH), np.float32)
        SIGc.reshape(NB, 32, BCH)[:, :NSTEP, :] = (
            sig[steps].reshape(NSTEP, NB, BCH).transpose(1, 0, 2)
        )
        B3Dc = np.zeros((128, 1), np.float32)
        B3Dc.reshape(NB, 32)[:, :NSTEP] = b3d[steps][None, :]

        in_maps.append({
            "S": S,
            "W1": np.ascontiguousarray(W1m[steps]),
            "W2": W2c,
            "B1": B1c,
            "B2": B2c,
            "W3D": W3Dc,
            "B3D": B3Dc,
            "SIG": SIGc,
        })
    return in_maps


def kernel(samples, W1, b1, W2, b2, W3, b3):
    global LAST_RESULT
    nc = _get_graph()
    in_maps = _prep_inputs(samples, W1, b1, W2, b2, W3, b3)
    res = run_bass_kernel_spmd(
        nc, in_maps, core_ids=list(range(NCORES)), trace=TRACE,
    )
    LAST_RESULT = res
    # out rows 32*b + j hold ln(1+exp(-x)) = -logp for (step j, batch chunk b)
    acc = np.zeros(B, np.float64)
    for c in range(NCORES):
        o = np.asarray(res.results[c]["out"], np.float64)  # [128, 512]
        per_step = o.reshape(NB, 32, BCH)[:, :NSTEP, :]    # (b, j, q)
        acc += per_step.transpose(1, 0, 2).reshape(NSTEP, B).sum(axis=0)
    return (-acc).astype(np.float32).reshape(1, B)


# revision 17
# speedup vs baseline: 1.3122x; 1.3122x over previous
"""Trainium2 Bass kernel: autoregressive wavefunction log-prob (N=64, B=2048, H=512).

Sharding: step axis N across 8 cores, round-robin (core c owns global steps
c, c+8, ..., c+56).  Each core computes, for its 8 steps i:
    h1 = relu(X_i @ W1_i + b1_i)        X_i = prefix one-hots (masked into W1)
    h2 = relu(h1 @ W2_i + b2_i)
    d  = h2 @ (W3_i[:,0]-W3_i[:,1]) + (b3_i[0]-b3_i[1])
    logp_i = ln(sigmoid(sigma_i * d))   sigma = s0 - s1 in {+1,-1}
and returns a [128, 512] tile holding -logp for (step j, batch-chunk b) at
row 32*b + j; the host gathers, negates and sums over steps/cores.

On-chip layout: activations stay transposed [H, B] so W1 [2N,H] and
W2 [Hin,Hout] serve directly as matmul lhsT.  The ragged prefix (rows >= 2i
of the padded W1) is zeroed on the host so a single SPMD graph serves all
cores.  Layer 1 runs in bf16; layer 2 in fp8e4m3 with DoubleRow (K=256 per
matmul).  Power-of-2 scale folding keeps every cast exact and free:
W1,b1 x8 (h1' = 8*h1 sits in fp8's sweet spot), W2 x16 fp8, b2 x128
(h2' = 128*h2 in bf16), W3D /128.  The scalar collapse d uses an M=8 matmul
whose lhsT has w3d in column j and zeros elsewhere, accumulated across all
steps into one PSUM bank via column-group tiling (tile_position=(0,32b)).
ln(sigmoid(x)) = -ln(1+exp(-x)) so Relu/Exp/Ln share one ACT table set.
Layer 1 of step j+1 is software-pipelined into step j's L2 loop to smooth
PSUM-slot reuse across the ACT/DVE epilogue queues.
"""

import numpy as np
import ml_dtypes

import concourse.bass as bass
import concourse.mybir as mybir
import concourse.tile as tile
from concourse.bass_utils import run_bass_kernel_spmd

N, B, H = 64, 2048, 512
NCORES = 8
NSTEP = N // NCORES          # 8 local steps per core
BCH = 512                    # batch chunk (one fp32 PSUM bank)
NB = B // BCH                # 4
NM = H // 128                # 4 h-chunks
K2N = 2 * N                  # 128, layer-1 contraction
BH = 2 * BCH                 # 1024, epilogue granularity

BF = mybir.dt.bfloat16
F32 = mybir.dt.float32
FP8 = mybir.dt.float8e4
NPBF = ml_dtypes.bfloat16
NPF8 = ml_dtypes.float8_e4m3

SH1 = 8.0      # h1 scale (folded into W1, b1)
SW2 = 16.0     # W2 fp8 scale
SZ2 = SH1 * SW2  # 128; folded into b2 and out of W3D

TRACE = False
LAST_RESULT = None


def _thin_sem_incs(nc):
    """Drop PE-semaphore increments whose cumulative value nobody waits on
    (each then_inc serializes ~26ns at the EVT_SEM register); renumber the
    surviving waits.  PE executes in order, so the v-th increment firing
    implies all prior PE work retired.  Only touches semaphores updated
    exclusively by PE instructions with update_value 1."""
    blocks = [blk for fn in nc.m.functions for blk in fn.blocks]
    upd = {}
    waited = {}
    ok = {}
    for blk in blocks:
        for inst in blk.instructions:
            si = inst.sync_info
            if si is None:
                continue
            for w in (si.on_wait or []):
                if w.wait_mode == "sem-ge-imm" and w.wait_value is not None:
                    waited.setdefault(w.id, set()).add(w.wait_value)
                else:
                    ok[w.id] = False
            for u in (si.on_update or []):
                lst = upd.setdefault(u.id, [])
                lst.append((inst, (lst[-1][1] if lst else 0) + (u.update_value or 0)))
                is_pe = (
                    inst.engine == mybir.EngineType.PE
                    and u.update_value == 1
                    and getattr(u, "update_mode", "sem-inc") == "sem-inc"
                )
                if not is_pe:
                    ok[u.id] = False
    for sem_id, updates in upd.items():
        if not ok.get(sem_id, True):
            continue
        keep_vals = sorted(v for v in waited.get(sem_id, set()) if v > 0)
        remap = {}
        new_cum = 0
        ki = 0
        for inst, cum in updates:
            if ki < len(keep_vals) and cum >= keep_vals[ki]:
                while ki < len(keep_vals) and keep_vals[ki] <= cum:
                    remap[keep_vals[ki]] = new_cum + 1
                    ki += 1
                new_cum += 1
            else:
                si = inst.sync_info
                nu = [u for u in (si.on_update or []) if u.id != sem_id]
                inst.sync_info = mybir.SyncInfo(
                    on_wait=list(si.on_wait or []), on_update=nu
                )
        for blk in blocks:
            for inst in blk.instructions:
                si = inst.sync_info
                if si is None or not si.on_wait:
                    continue
                changed = False
                nw = []
                for w in si.on_wait:
                    if w.id == sem_id and w.wait_value and w.wait_value > 0:
                        nw.append(mybir.SyncWait(
                            sync_type=w.sync_type, id=w.id,
                            ant_name=w.ant_name, wait_mode=w.wait_mode,
                            wait_value=remap[w.wait_value],
                        ))
                        changed = True
                    else:
                        nw.append(w)
                if changed:
                    inst.sync_info = mybir.SyncInfo(
                        on_wait=nw, on_update=list(si.on_update or [])
                    )
    return nc


def _legalize_waits(nc):
    """This walrus build encodes at most ONE semaphore wait per instruction;
    spill extras onto EventSemaphore instructions inserted just before, in
    the same engine's FIFO stream — semantically identical."""
    for fn in nc.m.functions:
        for blk in fn.blocks:
            new = []
            for inst in blk.instructions:
                si = inst.sync_info
                if si is not None and si.on_wait is not None and len(si.on_wait) > 1:
                    waits = list(si.on_wait)
                    for idx, w in enumerate(waits[:-1]):
                        new.append(mybir.InstEventSemaphore(
                            name=f"{inst.name}-spill{idx}",
                            engine=inst.engine,
                            sync_info=mybir.SyncInfo(on_wait=[w], on_update=[]),
                        ))
                    inst.sync_info = mybir.SyncInfo(
                        on_wait=[waits[-1]], on_update=list(si.on_update)
                    )
                new.append(inst)
            blk.instructions = new
    return nc


def build_graph():
    nc = bass.Bass()
    S_d = nc.declare_dram_parameter("S", [K2N, B], BF, False)
    W1_d = nc.declare_dram_parameter("W1", [NSTEP, K2N, H], BF, False)
    W2_d = nc.declare_dram_parameter("W2", [NSTEP, 128, NM * H], FP8, False)
    B1_d = nc.declare_dram_parameter("B1", [128, NSTEP * NM], F32, False)
    B2_d = nc.declare_dram_parameter("B2", [128, NSTEP * NM], F32, False)
    W3D_d = nc.declare_dram_parameter("W3D", [128, NSTEP * NM * NSTEP], BF, False)
    B3D_d = nc.declare_dram_parameter("B3D", [128, 1], F32, False)
    SIG_d = nc.declare_dram_parameter("SIG", [128, BCH], F32, False)
    OUT_d = nc.declare_dram_parameter("out", [128, BCH], F32, True)

    add = mybir.AluOpType.add
    amax = mybir.AluOpType.max
    mult = mybir.AluOpType.mult
    Relu = mybir.ActivationFunctionType.Relu
    Exp = mybir.ActivationFunctionType.Exp
    Ln = mybir.ActivationFunctionType.Ln
    DR = mybir.MatmulPerfMode.DoubleRow

    with tile.TileContext(nc) as tc:
        with (
            tc.tile_pool(name="const", bufs=1) as const,
            tc.tile_pool(name="w1p", bufs=3) as w1p,
            tc.tile_pool(name="w2p", bufs=3) as w2p,
            tc.tile_pool(name="h1p", bufs=5) as h1p,
            tc.tile_pool(name="h2p", bufs=7) as h2p,
            tc.tile_pool(name="tailp", bufs=2) as tailp,
            tc.tile_pool(name="pp", bufs=3, space="PSUM") as pp,
            tc.tile_pool(name="dp", bufs=1, space="PSUM") as dp,
        ):
            # ---- warmup: zeroed operands for PE HAM warmup + ACT table load
            wz = const.tile([128, BCH + 128], BF)
            nc.vector.memset(wz[:], 0.0)
            wact = const.tile([128, 1], F32)
            nc.vector.memset(wact[:], 0.0)
            nc.scalar.activation(wact[:], wact[:], Exp)
            nc.scalar.activation(wact[:], wact[:], Ln)
            nc.scalar.activation(wact[:], wact[:], Relu)

            # ---- startup DMAs, ordered by first use
            w1_first = w1p.tile([K2N, H], BF, tag="w1")
            nc.sync.dma_start(out=w1_first[:], in_=W1_d[0])
            S_sb = const.tile([K2N, B], BF)
            nc.sync.dma_start(out=S_sb[:], in_=S_d[:])
            B1_sb = const.tile([128, NSTEP * NM], F32)
            nc.sync.dma_start(out=B1_sb[:], in_=B1_d[:])
            w2_first = w2p.tile([128, NM, H], FP8, tag="w2")
            nc.sync.dma_start(
                out=w2_first[:],
                in_=W2_d[0].rearrange("p (k h) -> p k h", k=NM),
            )
            B2_sb = const.tile([128, NSTEP * NM], F32)
            nc.sync.dma_start(out=B2_sb[:], in_=B2_d[:])
            W3D_sb = const.tile([128, NSTEP * NM * NSTEP], BF)
            nc.sync.dma_start(out=W3D_sb[:], in_=W3D_d[:])
            B3D_sb = const.tile([128, 1], F32)
            nc.sync.dma_start(out=B3D_sb[:], in_=B3D_d[:])
            SIG_sb = const.tile([128, BCH], F32)
            nc.sync.dma_start(out=SIG_sb[:], in_=SIG_d[:])

            # persistent d accumulator: row 32*b + j = (batch chunk b, step j)
            D1 = dp.tile([128, BCH], F32)
            nc.vector.memset(D1[:], 0.0)

            # PE warmup on zeroed SBUF while DMAs land
            wps = pp.tile([128, BH], F32, tag="ps", name="wps")
            for _ in range(14):
                nc.tensor.matmul(
                    wps[:, 0:BCH], wz[:, BCH:BCH + 128], wz[:, 0:BCH],
                    start=True, stop=True, skip_group_check=True,
                )

            # ---------- emit helpers ----------
            def alloc_h1(j):
                # kk in {0,1}: [p, r, q] = h1'[(2kk+r)*128 + p, q]  (fp8)
                return [
                    h1p.tile([128, 2, B], FP8, tag="h1", name=f"h1_{j}_{kk}")
                    for kk in range(2)
                ]

            def emit_l1_block(j, m, w1, h1t):
                """Layer-1 h-chunk m of step j: h1' = relu(8(X@W1) + 8 b1)."""
                bias = B1_sb[:, j * NM + m: j * NM + m + 1]
                for half in range(2):
                    ps = pp.tile([128, BH], F32, tag="ps", name="ps")
                    for bsub in range(2):
                        b = 2 * half + bsub
                        nc.tensor.matmul(
                            ps[:, bsub * BCH:(bsub + 1) * BCH],
                            w1[:, m * 128:(m + 1) * 128],
                            S_sb[:, b * BCH:(b + 1) * BCH],
                            start=True, stop=True,
                        )
                    dst = h1t[m // 2][:, m % 2, half * BH:(half + 1) * BH]
                    if (m + half) % 2 == 0:
                        nc.scalar.activation(dst, ps[:], Relu, bias=bias)
                    else:
                        nc.vector.tensor_scalar(
                            dst, ps[:], bias, 0.0, op0=add, op1=amax,
                        )

            def emit_l2_block(j, m2, w2, h1t, h2):
                """Layer-2 out-chunk m2 (fp8 DoubleRow, K=256/mm):
                h2' = relu(z' + 128 b2), z' accumulated over kk."""
                bias = B2_sb[:, j * NM + m2: j * NM + m2 + 1]
                h2m = h2p.tile([128, B], BF, tag="h2", name=f"h2_{j}_{m2}")
                for half in range(2):
                    ps = pp.tile([128, BH], F32, tag="ps", name="ps")
                    for kk in range(2):
                        lhsT = w2[:, 2 * kk:2 * kk + 2, m2 * 128:(m2 + 1) * 128]
                        for bsub in range(2):
                            b = 2 * half + bsub
                            nc.tensor.matmul(
                                ps[:, bsub * BCH:(bsub + 1) * BCH],
                                lhsT,
                                h1t[kk][:, :, b * BCH:(b + 1) * BCH],
                                start=(kk == 0), stop=(kk == 1),
                                perf_mode=DR,
                            )
                    dst = h2m[:, half * BH:(half + 1) * BH]
                    if (m2 + half) % 2 == 0:
                        nc.scalar.activation(dst, ps[:], Relu, bias=bias)
                    else:
                        nc.vector.tensor_scalar(
                            dst, ps[:], bias, 0.0, op0=add, op1=amax,
                        )
                h2[m2] = h2m

            def emit_l3(j, k, h2):
                """D1[32b+j, :] += w3d_j^T @ h2'[k-chunk][b] (column groups)."""
                c0 = (j * NM + k) * NSTEP
                lhsT = W3D_sb[:, c0: c0 + NSTEP]
                for b in range(NB):
                    nc.tensor.matmul(
                        D1[32 * b: 32 * b + NSTEP, :],
                        lhsT,
                        h2[k][:, b * BCH:(b + 1) * BCH],
                        start=(j == 0 and k == 0),
                        stop=(j == NSTEP - 1 and k == NM - 1),
                        skip_group_check=True,
                        tile_position=(0, 32 * b),
                    )

            # ---------- main pipeline ----------
            # prologue: layer 1 of step 0
            h1_cur = alloc_h1(0)
            for m in range(NM):
                emit_l1_block(0, m, w1_first, h1_cur)

            w2 = w2_first
            prev_h2 = None
            for j in range(NSTEP):
                if j + 1 < NSTEP:
                    w1n = w1p.tile([K2N, H], BF, tag="w1", name=f"w1_{j+1}")
                    nc.sync.dma_start(out=w1n[:], in_=W1_d[j + 1])
                    w2n = w2p.tile([128, NM, H], FP8, tag="w2", name=f"w2_{j+1}")
                    nc.sync.dma_start(
                        out=w2n[:],
                        in_=W2_d[j + 1].rearrange("p (k h) -> p k h", k=NM),
                    )
                    h1_next = alloc_h1(j + 1)
                else:
                    w1n = w2n = h1_next = None

                h2 = {}
                for m2 in range(NM):
                    emit_l2_block(j, m2, w2, h1_cur, h2)
                    if h1_next is not None:
                        emit_l1_block(j + 1, m2, w1n, h1_next)
                    if m2 == 0 and prev_h2 is not None:
                        emit_l3(j - 1, NM - 1, prev_h2)  # deferred last chunk

                for k in range(NM - 1):
                    emit_l3(j, k, h2)

                prev_h2 = h2
                h1_cur = h1_next
                w2 = w2n
            emit_l3(NSTEP - 1, NM - 1, prev_h2)

            # ---- tail: -logp = ln(1 + exp(-sigma*(d + b3d))), one bank wide
            tt = tailp.tile([128, BCH], F32, tag="tt")
            nc.vector.scalar_tensor_tensor(
                tt[:], D1[:], B3D_sb[:, 0:1], SIG_sb[:], op0=add, op1=mult,
            )
            ex = tailp.tile([128, BCH], F32, tag="ex")
            nc.scalar.activation(ex[:], tt[:], Exp, scale=-1.0)
            e1 = tailp.tile([128, BCH], F32, tag="e1")
            nc.vector.tensor_scalar(e1[:], ex[:], 1.0, None, op0=add)
            lp = tailp.tile([128, BCH], F32, tag="lp")
            nc.scalar.activation(lp[:], e1[:], Ln)
            nc.sync.dma_start(out=OUT_d[:], in_=lp[:])

    return _legalize_waits(_thin_sem_incs(nc))


_NC_CACHE = None


def _get_graph():
    global _NC_CACHE
    if _NC_CACHE is None:
        _NC_CACHE = build_graph()
    return _NC_CACHE


def _prep_inputs(samples, W1, b1, W2, b2, W3, b3):
    samples = np.asarray(samples, np.float32)
    W1 = np.asarray(W1, np.float32)
    b1 = np.asarray(b1, np.float32)
    W2 = np.asarray(W2, np.float32)
    b2 = np.asarray(b2, np.float32)
    W3 = np.asarray(W3, np.float32)
    b3 = np.asarray(b3, np.float32)

    # S[2j+s, b] = samples[j, b, s]
    S = samples.transpose(0, 2, 1).reshape(K2N, B).astype(NPBF)
    # mask padded rows (row k of W1[i] is dead unless k < 2i); fold in SH1
    row = np.arange(K2N)[None, :, None]
    step = np.arange(N)[:, None, None]
    W1m = np.where(row < 2 * step, W1 * SH1, 0.0).astype(NPBF)
    w3d = ((W3[:, :, 0] - W3[:, :, 1]) / SZ2).astype(np.float32)  # (N, H)
    b3d = (b3[:, 0] - b3[:, 1]).astype(np.float32)                # (N,)
    sig = (samples[:, :, 0] - samples[:, :, 1]).astype(np.float32)

    in_maps = []
    for c in range(NCORES):
        steps = c + NCORES * np.arange(NSTEP)
        W2c = (
            (W2[steps] * SW2)
            .reshape(NSTEP, NM, 128, H)
            .transpose(0, 2, 1, 3)
            .reshape(NSTEP, 128, NM * H)
            .astype(NPF8)
        )
        B1c = (
            (b1[steps] * SH1).reshape(NSTEP, NM, 128).transpose(2, 0, 1)
            .reshape(128, NSTEP * NM).astype(np.float32)
        )
        B2c = (
            (b2[steps] * SZ2).reshape(NSTEP, NM, 128).transpose(2, 0, 1)
            .reshape(128, NSTEP * NM).astype(np.float32)
        )
        # W3D[p, ((j*NM + k)*NSTEP) + jj] = w3d[steps[j], k*128+p] if jj == j
        W3Dc = np.zeros((128, NSTEP, NM, NSTEP), np.float32)
        for j in range(NSTEP):
            W3Dc[:, j, :, j] = w3d[steps[j]].reshape(NM, 128).T
        W3Dc = W3Dc.reshape(128, NSTEP * NM * NSTEP).astype(NPBF)

        # row 32*b + j layouts for the tail
        SIGc = np.zeros((128, BCH), np.float32)
        SIGc.reshape(NB, 32, BCH)[:, :NSTEP, :] = (
            sig[steps].reshape(NSTEP, NB, BCH).transpose(1, 0, 2)
        )
        B3Dc = np.zeros((128, 1), np.float32)
        B3Dc.reshape(NB, 32)[:, :NSTEP] = b3d[steps][None, :]

        in_maps.append({
            "S": S,
            "W1": np.ascontiguousarray(W1m[steps]),
            "W2": W2c,
            "B1": B1c,
            "B2": B2c,
            "W3D": W3Dc,
            "B3D": B3Dc,
            "SIG": SIGc,
        })
    return in_maps


def kernel(samples, W1, b1, W2, b2, W3, b3):
    global LAST_RESULT
    nc = _get_graph()
    in_maps = _prep_inputs(samples, W1, b1, W2, b2, W3, b3)
    res = run_bass_kernel_spmd(
        nc, in_maps, core_ids=list(range(NCORES)), trace=TRACE,
    )
    LAST_RESULT = res
    # out rows 32*b + j hold -logp for (step j, batch chunk b)
    acc = np.zeros(B, np.float64)
    for c in range(NCORES):
        o = np.asarray(res.results[c]["out"], np.float64)  # [128, 512]
        per_step = o.reshape(NB, 32, BCH)[:, :NSTEP, :]    # (b, j, q)
        acc += per_step.transpose(1, 0, 2).reshape(NSTEP, B).sum(axis=0)
    return (-acc).astype(np.float32).reshape(1, B)


# revision 22
# speedup vs baseline: 1.5427x; 1.1757x over previous
"""Trainium2 Bass kernel: autoregressive wavefunction log-prob (N=64, B=2048, H=512).

Sharding: step axis N across 8 cores, round-robin (core c owns global steps
c, c+8, ..., c+56).  Each core computes, for its 8 steps i:
    h1 = relu(X_i @ W1_i + b1_i)        X_i = prefix one-hots (masked into W1)
    h2 = relu(h1 @ W2_i + b2_i)
    d  = h2 @ (W3_i[:,0]-W3_i[:,1]) + (b3_i[0]-b3_i[1])
    logp_i = ln(sigmoid(sigma_i * d))   sigma = s0 - s1 in {+1,-1}
and returns a [128, 512] tile holding -logp for (step j, batch-chunk b) at
row 32*b + j; the host gathers, negates and sums over steps/cores.

On-chip layout: activations stay transposed [H, B] so W1 [2N,H] and
W2 [Hin,Hout] serve directly as matmul lhsT.  The ragged prefix (rows >= 2i
of the padded W1) is zeroed on the host so a single SPMD graph serves all
cores.  Layer 1 runs in bf16; layer 2 in fp8e4m3 with DoubleRow (K=256 per
matmul).  Power-of-2 scale folding keeps every cast exact and free:
W1,b1 x8 (h1' = 8*h1 sits in fp8's sweet spot), W2 x16 fp8, b2 x128
(h2' = 128*h2 in bf16), W3D /128.  The scalar collapse d uses an M=8 matmul
whose lhsT has w3d in column j and zeros elsewhere, accumulated across all
steps into one PSUM bank via column-group tiling (tile_position=(0,32b)).
ln(sigmoid(x)) = -ln(1+exp(-x)) so Relu/Exp/Ln share one ACT table set.
Layer 1 of step j+1 is software-pipelined into step j's L2 loop to smooth
PSUM-slot reuse across the ACT/DVE epilogue queues.
"""

import numpy as np
import ml_dtypes

import concourse.bass as bass
import concourse.mybir as mybir
import concourse.tile as tile
from concourse.bass_utils import run_bass_kernel_spmd

N, B, H = 64, 2048, 512
NCORES = 8
NSTEP = N // NCORES          # 8 local steps per core
BCH = 512                    # batch chunk (one fp32 PSUM bank)
NB = B // BCH                # 4
NM = H // 128                # 4 h-chunks
K2N = 2 * N                  # 128, layer-1 contraction
BH = 2 * BCH                 # 1024, epilogue granularity

BF = mybir.dt.bfloat16
F32 = mybir.dt.float32
FP8 = mybir.dt.float8e4
NPBF = ml_dtypes.bfloat16
NPF8 = ml_dtypes.float8_e4m3

SH1 = 8.0      # h1 scale (folded into W1, b1)
SW2 = 16.0     # W2 fp8 scale
SZ2 = SH1 * SW2  # 128; folded into b2 and out of W3D

TRACE = False
LAST_RESULT = None


def _thin_sem_incs(nc):
    """Drop PE-semaphore increments whose cumulative value nobody waits on
    (each then_inc serializes ~26ns at the EVT_SEM register); renumber the
    surviving waits.  PE executes in order, so the v-th increment firing
    implies all prior PE work retired.  Only touches semaphores updated
    exclusively by PE instructions with update_value 1."""
    blocks = [blk for fn in nc.m.functions for blk in fn.blocks]
    upd = {}
    waited = {}
    ok = {}
    for blk in blocks:
        for inst in blk.instructions:
            si = inst.sync_info
            if si is None:
                continue
            for w in (si.on_wait or []):
                if w.wait_mode == "sem-ge-imm" and w.wait_value is not None:
                    waited.setdefault(w.id, set()).add(w.wait_value)
                else:
                    ok[w.id] = False
            for u in (si.on_update or []):
                lst = upd.setdefault(u.id, [])
                lst.append((inst, (lst[-1][1] if lst else 0) + (u.update_value or 0)))
                is_pe = (
                    inst.engine == mybir.EngineType.PE
                    and u.update_value == 1
                    and getattr(u, "update_mode", "sem-inc") == "sem-inc"
                )
                if not is_pe:
                    ok[u.id] = False
    for sem_id, updates in upd.items():
        if not ok.get(sem_id, True):
            continue
        keep_vals = sorted(v for v in waited.get(sem_id, set()) if v > 0)
        remap = {}
        new_cum = 0
        ki = 0
        for inst, cum in updates:
            if ki < len(keep_vals) and cum >= keep_vals[ki]:
                while ki < len(keep_vals) and keep_vals[ki] <= cum:
                    remap[keep_vals[ki]] = new_cum + 1
                    ki += 1
                new_cum += 1
            else:
                si = inst.sync_info
                nu = [u for u in (si.on_update or []) if u.id != sem_id]
                inst.sync_info = mybir.SyncInfo(
                    on_wait=list(si.on_wait or []), on_update=nu
                )
        for blk in blocks:
            for inst in blk.instructions:
                si = inst.sync_info
                if si is None or not si.on_wait:
                    continue
                changed = False
                nw = []
                for w in si.on_wait:
                    if w.id == sem_id and w.wait_value and w.wait_value > 0:
                        nw.append(mybir.SyncWait(
                            sync_type=w.sync_type, id=w.id,
                            ant_name=w.ant_name, wait_mode=w.wait_mode,
                            wait_value=remap[w.wait_value],
                        ))
                        changed = True
                    else:
                        nw.append(w)
                if changed:
                    inst.sync_info = mybir.SyncInfo(
                        on_wait=nw, on_update=list(si.on_update or [])
                    )
    return nc


def _elide_redundant_ldweights(nc):
    """Tile lowers every matmul into an Ldweights+Matmult pair.  Consecutive
    matmuls sharing the stationary operand don't need the repeated loads (the
    PE weight registers are untouched by intervening Matmults).  A dropped
    load's semaphore waits are kept as an EventSemaphore in the PE stream."""
    for fn in nc.m.functions:
        for blk in fn.blocks:
            new = []
            last_key = None
            for inst in blk.instructions:
                if isinstance(inst, mybir.InstMatmult):
                    new.append(inst)
                    continue
                if isinstance(inst, mybir.InstLdweights):
                    a = inst.ins[0]
                    key = (
                        a.memref, a.offset, str(a.ap), str(a.dtype),
                        str(inst.perf_mode), str(inst.tile_position),
                        str(inst.is_transpose),
                    )
                    if key == last_key:
                        si = inst.sync_info
                        if si is not None and (si.on_wait or si.on_update):
                            new.append(mybir.InstEventSemaphore(
                                name=f"{inst.name}-ldwelide",
                                engine=inst.engine,
                                sync_info=si,
                            ))
                        continue
                    last_key = key
                    new.append(inst)
                    continue
                if inst.engine == mybir.EngineType.PE:
                    last_key = None
                new.append(inst)
            blk.instructions = new
    return nc


def _legalize_waits(nc):
    """This walrus build encodes at most ONE semaphore wait per instruction;
    spill extras onto EventSemaphore instructions inserted just before, in
    the same engine's FIFO stream — semantically identical."""
    for fn in nc.m.functions:
        for blk in fn.blocks:
            new = []
            for inst in blk.instructions:
                si = inst.sync_info
                if si is not None and si.on_wait is not None and len(si.on_wait) > 1:
                    waits = list(si.on_wait)
                    for idx, w in enumerate(waits[:-1]):
                        new.append(mybir.InstEventSemaphore(
                            name=f"{inst.name}-spill{idx}",
                            engine=inst.engine,
                            sync_info=mybir.SyncInfo(on_wait=[w], on_update=[]),
                        ))
                    inst.sync_info = mybir.SyncInfo(
                        on_wait=[waits[-1]], on_update=list(si.on_update)
                    )
                new.append(inst)
            blk.instructions = new
    return nc


def build_graph():
    nc = bass.Bass()
    S_d = nc.declare_dram_parameter("S", [K2N, B], BF, False)
    W1_d = nc.declare_dram_parameter("W1", [NSTEP, K2N, H], BF, False)
    W2_d = nc.declare_dram_parameter("W2", [NSTEP, 128, NM * H], FP8, False)
    B1_d = nc.declare_dram_parameter("B1", [128, NSTEP * NM], F32, False)
    B2_d = nc.declare_dram_parameter("B2", [128, NSTEP * NM], F32, False)
    W3D_d = nc.declare_dram_parameter("W3D", [128, NSTEP * NM * NSTEP], BF, False)
    B3D_d = nc.declare_dram_parameter("B3D", [128, 1], F32, False)
    SIG_d = nc.declare_dram_parameter("SIG", [128, BCH], F32, False)
    OUT_d = nc.declare_dram_parameter("out", [128, BCH], F32, True)

    add = mybir.AluOpType.add
    amax = mybir.AluOpType.max
    mult = mybir.AluOpType.mult
    Relu = mybir.ActivationFunctionType.Relu
    Exp = mybir.ActivationFunctionType.Exp
    Ln = mybir.ActivationFunctionType.Ln
    DR = mybir.MatmulPerfMode.DoubleRow

    with tile.TileContext(nc) as tc:
        with (
            tc.tile_pool(name="const", bufs=1) as const,
            tc.tile_pool(name="w1p", bufs=3) as w1p,
            tc.tile_pool(name="w2p", bufs=3) as w2p,
            tc.tile_pool(name="h1p", bufs=5) as h1p,
            tc.tile_pool(name="h2p", bufs=7) as h2p,
            tc.tile_pool(name="tailp", bufs=2) as tailp,
            tc.tile_pool(name="pp", bufs=3, space="PSUM") as pp,
            tc.tile_pool(name="dp", bufs=1, space="PSUM") as dp,
        ):
            # ---- warmup: zeroed operands for PE HAM warmup + ACT table load
            wz = const.tile([128, BCH + 128], BF)
            nc.vector.memset(wz[:], 0.0)
            wact = const.tile([128, 1], F32)
            nc.vector.memset(wact[:], 0.0)
            nc.scalar.activation(wact[:], wact[:], Exp)
            nc.scalar.activation(wact[:], wact[:], Ln)
            nc.scalar.activation(wact[:], wact[:], Relu)

            # ---- startup DMAs, ordered by first use
            w1_first = w1p.tile([K2N, H], BF, tag="w1")
            nc.sync.dma_start(out=w1_first[:], in_=W1_d[0])
            S_sb = const.tile([K2N, B], BF)
            nc.sync.dma_start(out=S_sb[:, 0:BH], in_=S_d[:, 0:BH])
            nc.sync.dma_start(out=S_sb[:, BH:B], in_=S_d[:, BH:B])
            B1_sb = const.tile([128, NSTEP * NM], F32)
            nc.sync.dma_start(out=B1_sb[:], in_=B1_d[:])
            w2_first = w2p.tile([128, NM, H], FP8, tag="w2")
            nc.sync.dma_start(
                out=w2_first[:],
                in_=W2_d[0].rearrange("p (k h) -> p k h", k=NM),
            )
            B2_sb = const.tile([128, NSTEP * NM], F32)
            nc.sync.dma_start(out=B2_sb[:], in_=B2_d[:])
            W3D_sb = const.tile([128, NSTEP * NM * NSTEP], BF)
            nc.sync.dma_start(out=W3D_sb[:], in_=W3D_d[:])
            B3D_sb = const.tile([128, 1], F32)
            nc.sync.dma_start(out=B3D_sb[:], in_=B3D_d[:])
            SIG_sb = const.tile([128, BCH], F32)
            nc.sync.dma_start(out=SIG_sb[:], in_=SIG_d[:])

            # persistent d accumulator: row 32*b + j = (batch chunk b, step j)
            D1 = dp.tile([128, BCH], F32)
            nc.vector.memset(D1[:], 0.0)

            # PE warmup on zeroed SBUF while DMAs land (read wps afterwards so
            # the matmuls aren't dead code)
            wps = pp.tile([128, BH], F32, tag="ps", name="wps")
            for _ in range(14):
                nc.tensor.matmul(
                    wps[:, 0:BCH], wz[:, BCH:BCH + 128], wz[:, 0:BCH],
                    start=True, stop=True, skip_group_check=True,
                )
            nc.vector.tensor_copy(wact[:], wps[:, 0:1])

            # ---------- emit helpers ----------
            def alloc_h1(j):
                # kk in {0,1}: [p, r, q] = h1'[(2kk+r)*128 + p, q]  (fp8)
                return [
                    h1p.tile([128, 2, B], FP8, tag="h1", name=f"h1_{j}_{kk}")
                    for kk in range(2)
                ]

            def emit_l1_block(j, m, w1, h1t):
                """Layer-1 h-chunk m of step j: h1' = relu(8(X@W1) + 8 b1)."""
                bias = B1_sb[:, j * NM + m: j * NM + m + 1]
                for half in range(2):
                    ps = pp.tile([128, BH], F32, tag="ps", name="ps")
                    for bsub in range(2):
                        b = 2 * half + bsub
                        nc.tensor.matmul(
                            ps[:, bsub * BCH:(bsub + 1) * BCH],
                            w1[:, m * 128:(m + 1) * 128],
                            S_sb[:, b * BCH:(b + 1) * BCH],
                            start=True, stop=True,
                        )
                    dst = h1t[m // 2][:, m % 2, half * BH:(half + 1) * BH]
                    if (m + half) % 2 == 0:
                        nc.scalar.activation(dst, ps[:], Relu, bias=bias)
                    else:
                        nc.vector.tensor_scalar(
                            dst, ps[:], bias, 0.0, op0=add, op1=amax,
                        )

            def emit_l2_block(j, m2, w2, h1t, h2):
                """Layer-2 out-chunk m2 (fp8 DoubleRow, K=256/mm):
                h2' = relu(z' + 128 b2), z' accumulated over kk."""
                bias = B2_sb[:, j * NM + m2: j * NM + m2 + 1]
                h2m = h2p.tile([128, B], BF, tag="h2", name=f"h2_{j}_{m2}")
                psh = [
                    pp.tile([128, BH], F32, tag="ps", name=f"psh_{j}_{m2}_{h}")
                    for h in range(2)
                ]
                for kk in range(2):  # kk outer: one weight load per (m2, kk)
                    lhsT = w2[:, 2 * kk:2 * kk + 2, m2 * 128:(m2 + 1) * 128]
                    for half in range(2):
                        for bsub in range(2):
                            b = 2 * half + bsub
                            nc.tensor.matmul(
                                psh[half][:, bsub * BCH:(bsub + 1) * BCH],
                                lhsT,
                                h1t[kk][:, :, b * BCH:(b + 1) * BCH],
                                start=(kk == 0), stop=(kk == 1),
                                perf_mode=DR,
                            )
                for half in range(2):
                    dst = h2m[:, half * BH:(half + 1) * BH]
                    if (m2 + half) % 2 == 0:
                        nc.scalar.activation(dst, psh[half][:], Relu, bias=bias)
                    else:
                        nc.vector.tensor_scalar(
                            dst, psh[half][:], bias, 0.0, op0=add, op1=amax,
                        )
                h2[m2] = h2m

            def emit_l3(j, k, h2):
                """D1[32b+j, :] += w3d_j^T @ h2'[k-chunk][b] (column groups)."""
                c0 = (j * NM + k) * NSTEP
                lhsT = W3D_sb[:, c0: c0 + NSTEP]
                for b in range(NB):
                    nc.tensor.matmul(
                        D1[32 * b: 32 * b + NSTEP, :],
                        lhsT,
                        h2[k][:, b * BCH:(b + 1) * BCH],
                        start=(j == 0 and k == 0),
                        stop=(j == NSTEP - 1 and k == NM - 1),
                        skip_group_check=True,
                        tile_position=(0, 32 * b),
                    )

            # ---------- main pipeline ----------
            # prologue: layer 1 of step 0
            h1_cur = alloc_h1(0)
            for m in range(NM):
                emit_l1_block(0, m, w1_first, h1_cur)

            w2 = w2_first
            prev_h2 = None
            for j in range(NSTEP):
                if j + 1 < NSTEP:
                    w1n = w1p.tile([K2N, H], BF, tag="w1", name=f"w1_{j+1}")
                    nc.sync.dma_start(out=w1n[:], in_=W1_d[j + 1])
                    w2n = w2p.tile([128, NM, H], FP8, tag="w2", name=f"w2_{j+1}")
                    nc.sync.dma_start(
                        out=w2n[:],
                        in_=W2_d[j + 1].rearrange("p (k h) -> p k h", k=NM),
                    )
                    h1_next = alloc_h1(j + 1)
                else:
                    w1n = w2n = h1_next = None

                h2 = {}
                for m2 in range(NM):
                    emit_l2_block(j, m2, w2, h1_cur, h2)
                    if h1_next is not None:
                        emit_l1_block(j + 1, m2, w1n, h1_next)
                    if m2 == 0 and prev_h2 is not None:
                        emit_l3(j - 1, NM - 1, prev_h2)  # deferred last chunk

                for k in range(NM - 1):
                    emit_l3(j, k, h2)

                prev_h2 = h2
                h1_cur = h1_next
                w2 = w2n
            emit_l3(NSTEP - 1, NM - 1, prev_h2)

            # ---- tail: -logp = ln(1 + exp(-sigma*(d + b3d))), one bank wide
            tt = tailp.tile([128, BCH], F32, tag="tt")
            nc.vector.scalar_tensor_tensor(
                tt[:], D1[:], B3D_sb[:, 0:1], SIG_sb[:], op0=add, op1=mult,
            )
            ex = tailp.tile([128, BCH], F32, tag="ex")
            nc.scalar.activation(ex[:], tt[:], Exp, scale=-1.0)
            e1 = tailp.tile([128, BCH], F32, tag="e1")
            nc.vector.tensor_scalar(e1[:], ex[:], 1.0, None, op0=add)
            lp = tailp.tile([128, BCH], F32, tag="lp")
            nc.scalar.activation(lp[:], e1[:], Ln)
            nc.sync.dma_start(out=OUT_d[:], in_=lp[:])

    return _legalize_waits(_thin_sem_incs(_elide_redundant_ldweights(nc)))


_NC_CACHE = None


def _get_graph():
    global _NC_CACHE
    if _NC_CACHE is None:
        _NC_CACHE = build_graph()
    return _NC_CACHE


def _prep_inputs(samples, W1, b1, W2, b2, W3, b3):
    samples = np.asarray(samples, np.float32)
    W1 = np.asarray(W1, np.float32)
    b1 = np.asarray(b1, np.float32)
    W2 = np.asarray(W2, np.float32)
    b2 = np.asarray(b2, np.float32)
    W3 = np.asarray(W3, np.float32)
    b3 = np.asarray(b3, np.float32)

    # S[2j+s, b] = samples[j, b, s]
    S = samples.transpose(0, 2, 1).reshape(K2N, B).astype(NPBF)
    # mask padded rows (row k of W1[i] is dead unless k < 2i); fold in SH1
    row = np.arange(K2N)[None, :, None]
    step = np.arange(N)[:, None, None]
    W1m = np.where(row < 2 * step, W1 * SH1, 0.0).astype(NPBF)
    w3d = ((W3[:, :, 0] - W3[:, :, 1]) / SZ2).astype(np.float32)  # (N, H)
    b3d = (b3[:, 0] - b3[:, 1]).astype(np.float32)                # (N,)
    sig = (samples[:, :, 0] - samples[:, :, 1]).astype(np.float32)

    in_maps = []
    for c in range(NCORES):
        steps = c + NCORES * np.arange(NSTEP)
        W2c = (
            (W2[steps] * SW2)
            .reshape(NSTEP, NM, 128, H)
            .transpose(0, 2, 1, 3)
            .reshape(NSTEP, 128, NM * H)
            .astype(NPF8)
        )
        B1c = (
            (b1[steps] * SH1).reshape(NSTEP, NM, 128).transpose(2, 0, 1)
            .reshape(128, NSTEP * NM).astype(np.float32)
        )
        B2c = (
            (b2[steps] * SZ2).reshape(NSTEP, NM, 128).transpose(2, 0, 1)
            .reshape(128, NSTEP * NM).astype(np.float32)
        )
        # W3D[p, ((j*NM + k)*NSTEP) + jj] = w3d[steps[j], k*128+p] if jj == j
        W3Dc = np.zeros((128, NSTEP, NM, NSTEP), np.float32)
        for j in range(NSTEP):
            W3Dc[:, j, :, j] = w3d[steps[j]].reshape(NM, 128).T
        W3Dc = W3Dc.reshape(128, NSTEP * NM * NSTEP).astype(NPBF)

        # row 32*b + j layouts for the tail
        SIGc = np.zeros((128, BCH), np.float32)
        SIGc.reshape(NB, 32, BCH)[:, :NSTEP, :] = (
            sig[steps].reshape(NSTEP, NB, BCH).transpose(1, 0, 2)
        )
        B3Dc = np.zeros((128, 1), np.float32)
        B3Dc.reshape(NB, 32)[:, :NSTEP] = b3d[steps][None, :]

        in_maps.append({
            "S": S,
            "W1": np.ascontiguousarray(W1m[steps]),
            "W2": W2c,
            "B1": B1c,
            "B2": B2c,
            "W3D": W3Dc,
            "B3D": B3Dc,
            "SIG": SIGc,
        })
    return in_maps


def kernel(samples, W1, b1, W2, b2, W3, b3):
    global LAST_RESULT
    nc = _get_graph()
    in_maps = _prep_inputs(samples, W1, b1, W2, b2, W3, b3)
    res = run_bass_kernel_spmd(
        nc, in_maps, core_ids=list(range(NCORES)), trace=TRACE,
    )
    LAST_RESULT = res
    # out rows 32*b + j hold -logp for (step j, batch chunk b)
    acc = np.zeros(B, np.float64)
    for c in range(NCORES):
        o = np.asarray(res.results[c]["out"], np.float64)  # [128, 512]
        per_step = o.reshape(NB, 32, BCH)[:, :NSTEP, :]    # (b, j, q)
        acc += per_step.transpose(1, 0, 2).reshape(NSTEP, B).sum(axis=0)
    return (-acc).astype(np.float32).reshape(1, B)


# revision 23
# speedup vs baseline: 1.5478x; 1.0033x over previous
"""Trainium2 Bass kernel: autoregressive wavefunction log-prob (N=64, B=2048, H=512).

Sharding: step axis N across 8 cores, round-robin (core c owns global steps
c, c+8, ..., c+56).  Each core computes, for its 8 steps i:
    h1 = relu(X_i @ W1_i + b1_i)        X_i = prefix one-hots (masked into W1)
    h2 = relu(h1 @ W2_i + b2_i)
    d  = h2 @ (W3_i[:,0]-W3_i[:,1]) + (b3_i[0]-b3_i[1])
    logp_i = ln(sigmoid(sigma_i * d))   sigma = s0 - s1 in {+1,-1}
and returns a [128, 512] tile holding -logp for (step j, batch-chunk b) at
row 32*b + j; the host gathers, negates and sums over steps/cores.

On-chip layout: activations stay transposed [H, B] so W1 [2N,H] and
W2 [Hin,Hout] serve directly as matmul lhsT.  The ragged prefix (rows >= 2i
of the padded W1) is zeroed on the host so a single SPMD graph serves all
cores.  Layer 1 runs in bf16; layer 2 in fp8e4m3 with DoubleRow (K=256 per
matmul).  Power-of-2 scale folding keeps every cast exact and free:
W1,b1 x8 (h1' = 8*h1 sits in fp8's sweet spot), W2 x16 fp8, b2 x128
(h2' = 128*h2 in bf16), W3D /128.  The scalar collapse d uses an M=8 matmul
whose lhsT has w3d in column j and zeros elsewhere, accumulated across all
steps into one PSUM bank via column-group tiling (tile_position=(0,32b)).
ln(sigmoid(x)) = -ln(1+exp(-x)) so Relu/Exp/Ln share one ACT table set.
Layer 1 of step j+1 is software-pipelined into step j's L2 loop to smooth
PSUM-slot reuse across the ACT/DVE epilogue queues.
"""

import numpy as np
import ml_dtypes

import concourse.bass as bass
import concourse.mybir as mybir
import concourse.tile as tile
from concourse.bass_utils import run_bass_kernel_spmd

N, B, H = 64, 2048, 512
NCORES = 8
NSTEP = N // NCORES          # 8 local steps per core
BCH = 512                    # batch chunk (one fp32 PSUM bank)
NB = B // BCH                # 4
NM = H // 128                # 4 h-chunks
K2N = 2 * N                  # 128, layer-1 contraction
BH = 2 * BCH                 # 1024, epilogue granularity

BF = mybir.dt.bfloat16
F32 = mybir.dt.float32
FP8 = mybir.dt.float8e4
NPBF = ml_dtypes.bfloat16
NPF8 = ml_dtypes.float8_e4m3

SH1 = 8.0      # h1 scale (folded into W1, b1)
SW2 = 16.0     # W2 fp8 scale
SZ2 = SH1 * SW2  # 128; folded into b2 and out of W3D

TRACE = False
LAST_RESULT = None


def _thin_sem_incs(nc):
    """Drop PE-semaphore increments whose cumulative value nobody waits on
    (each then_inc serializes ~26ns at the EVT_SEM register); renumber the
    surviving waits.  PE executes in order, so the v-th increment firing
    implies all prior PE work retired.  Only touches semaphores updated
    exclusively by PE instructions with update_value 1."""
    blocks = [blk for fn in nc.m.functions for blk in fn.blocks]
    upd = {}
    waited = {}
    ok = {}
    for blk in blocks:
        for inst in blk.instructions:
            si = inst.sync_info
            if si is None:
                continue
            for w in (si.on_wait or []):
                if w.wait_mode == "sem-ge-imm" and w.wait_value is not None:
                    waited.setdefault(w.id, set()).add(w.wait_value)
                else:
                    ok[w.id] = False
            for u in (si.on_update or []):
                lst = upd.setdefault(u.id, [])
                lst.append((inst, (lst[-1][1] if lst else 0) + (u.update_value or 0)))
                is_pe = (
                    inst.engine == mybir.EngineType.PE
                    and u.update_value == 1
                    and getattr(u, "update_mode", "sem-inc") == "sem-inc"
                )
                if not is_pe:
                    ok[u.id] = False
    for sem_id, updates in upd.items():
        if not ok.get(sem_id, True):
            continue
        keep_vals = sorted(v for v in waited.get(sem_id, set()) if v > 0)
        remap = {}
        new_cum = 0
        ki = 0
        for inst, cum in updates:
            if ki < len(keep_vals) and cum >= keep_vals[ki]:
                while ki < len(keep_vals) and keep_vals[ki] <= cum:
                    remap[keep_vals[ki]] = new_cum + 1
                    ki += 1
                new_cum += 1
            else:
                si = inst.sync_info
                nu = [u for u in (si.on_update or []) if u.id != sem_id]
                inst.sync_info = mybir.SyncInfo(
                    on_wait=list(si.on_wait or []), on_update=nu
                )
        for blk in blocks:
            for inst in blk.instructions:
                si = inst.sync_info
                if si is None or not si.on_wait:
                    continue
                changed = False
                nw = []
                for w in si.on_wait:
                    if w.id == sem_id and w.wait_value and w.wait_value > 0:
                        nw.append(mybir.SyncWait(
                            sync_type=w.sync_type, id=w.id,
                            ant_name=w.ant_name, wait_mode=w.wait_mode,
                            wait_value=remap[w.wait_value],
                        ))
                        changed = True
                    else:
                        nw.append(w)
                if changed:
                    inst.sync_info = mybir.SyncInfo(
                        on_wait=nw, on_update=list(si.on_update or [])
                    )
    return nc


def _elide_redundant_ldweights(nc):
    """Tile lowers every matmul into an Ldweights+Matmult pair.  Consecutive
    matmuls sharing the stationary operand don't need the repeated loads (the
    PE weight registers are untouched by intervening Matmults).  A dropped
    load's semaphore waits are kept as an EventSemaphore in the PE stream."""
    for fn in nc.m.functions:
        for blk in fn.blocks:
            new = []
            last_key = None
            for inst in blk.instructions:
                if isinstance(inst, mybir.InstMatmult):
                    new.append(inst)
                    continue
                if isinstance(inst, mybir.InstLdweights):
                    a = inst.ins[0]
                    key = (
                        a.memref, a.offset, str(a.ap), str(a.dtype),
                        str(inst.perf_mode), str(inst.tile_position),
                        str(inst.is_transpose),
                    )
                    if key == last_key:
                        si = inst.sync_info
                        if si is not None and (si.on_wait or si.on_update):
                            new.append(mybir.InstEventSemaphore(
                                name=f"{inst.name}-ldwelide",
                                engine=inst.engine,
                                sync_info=si,
                            ))
                        continue
                    last_key = key
                    new.append(inst)
                    continue
                if inst.engine == mybir.EngineType.PE:
                    last_key = None
                new.append(inst)
            blk.instructions = new
    return nc


def _legalize_waits(nc):
    """This walrus build encodes at most ONE semaphore wait per instruction;
    spill extras onto EventSemaphore instructions inserted just before, in
    the same engine's FIFO stream — semantically identical."""
    for fn in nc.m.functions:
        for blk in fn.blocks:
            new = []
            for inst in blk.instructions:
                si = inst.sync_info
                if si is not None and si.on_wait is not None and len(si.on_wait) > 1:
                    waits = list(si.on_wait)
                    for idx, w in enumerate(waits[:-1]):
                        new.append(mybir.InstEventSemaphore(
                            name=f"{inst.name}-spill{idx}",
                            engine=inst.engine,
                            sync_info=mybir.SyncInfo(on_wait=[w], on_update=[]),
                        ))
                    inst.sync_info = mybir.SyncInfo(
                        on_wait=[waits[-1]], on_update=list(si.on_update)
                    )
                new.append(inst)
            blk.instructions = new
    return nc


def build_graph():
    nc = bass.Bass()
    S_d = nc.declare_dram_parameter("S", [K2N, B], BF, False)
    W1_d = nc.declare_dram_parameter("W1", [NSTEP, K2N, H], BF, False)
    W2_d = nc.declare_dram_parameter("W2", [NSTEP, 128, NM * H], FP8, False)
    B1_d = nc.declare_dram_parameter("B1", [128, NSTEP * NM], F32, False)
    B2_d = nc.declare_dram_parameter("B2", [128, NSTEP * NM], F32, False)
    W3D_d = nc.declare_dram_parameter("W3D", [128, NSTEP * NM * NSTEP], BF, False)
    B3D_d = nc.declare_dram_parameter("B3D", [128, 1], F32, False)
    SIG_d = nc.declare_dram_parameter("SIG", [128, BCH], F32, False)
    OUT_d = nc.declare_dram_parameter("out", [128, BCH], F32, True)

    add = mybir.AluOpType.add
    amax = mybir.AluOpType.max
    mult = mybir.AluOpType.mult
    Relu = mybir.ActivationFunctionType.Relu
    Exp = mybir.ActivationFunctionType.Exp
    Ln = mybir.ActivationFunctionType.Ln
    DR = mybir.MatmulPerfMode.DoubleRow

    with tile.TileContext(nc) as tc:
        with (
            tc.tile_pool(name="const", bufs=1) as const,
            tc.tile_pool(name="w1p", bufs=3) as w1p,
            tc.tile_pool(name="w2p", bufs=3) as w2p,
            tc.tile_pool(name="h1p", bufs=5) as h1p,
            tc.tile_pool(name="h2p", bufs=7) as h2p,
            tc.tile_pool(name="tailp", bufs=2) as tailp,
            tc.tile_pool(name="pp", bufs=3, space="PSUM") as pp,
            tc.tile_pool(name="dp", bufs=1, space="PSUM") as dp,
        ):
            # ---- warmup: zeroed operands for PE HAM warmup + ACT table load
            wz = const.tile([128, BCH + 128], BF)
            nc.vector.memset(wz[:], 0.0)
            wact = const.tile([128, 1], F32)
            nc.vector.memset(wact[:], 0.0)
            nc.scalar.activation(wact[:], wact[:], Exp)
            nc.scalar.activation(wact[:], wact[:], Ln)
            nc.scalar.activation(wact[:], wact[:], Relu)

            # ---- startup DMAs, ordered by first use
            w1_first = w1p.tile([K2N, H], BF, tag="w1")
            nc.sync.dma_start(out=w1_first[:], in_=W1_d[0])
            S_sb = const.tile([K2N, B], BF)
            nc.sync.dma_start(out=S_sb[:, 0:BH], in_=S_d[:, 0:BH])
            nc.sync.dma_start(out=S_sb[:, BH:B], in_=S_d[:, BH:B])
            B1_sb = const.tile([128, NSTEP * NM], F32)
            nc.sync.dma_start(out=B1_sb[:], in_=B1_d[:])
            w2_first = w2p.tile([128, NM, H], FP8, tag="w2")
            nc.sync.dma_start(
                out=w2_first[:],
                in_=W2_d[0].rearrange("p (k h) -> p k h", k=NM),
            )
            B2_sb = const.tile([128, NSTEP * NM], F32)
            nc.sync.dma_start(out=B2_sb[:], in_=B2_d[:])
            W3D_sb = const.tile([128, NSTEP * NM * NSTEP], BF)
            nc.sync.dma_start(out=W3D_sb[:], in_=W3D_d[:])
            B3D_sb = const.tile([128, 1], F32)
            nc.sync.dma_start(out=B3D_sb[:], in_=B3D_d[:])
            SIG_sb = const.tile([128, BCH], F32)
            nc.sync.dma_start(out=SIG_sb[:], in_=SIG_d[:])

            # persistent d accumulator: row 32*b + j = (batch chunk b, step j)
            D1 = dp.tile([128, BCH], F32)
            nc.vector.memset(D1[:], 0.0)

            # PE warmup on zeroed SBUF while DMAs land (read wps afterwards so
            # the matmuls aren't dead code)
            wps = pp.tile([128, BH], F32, tag="ps", name="wps")
            for _ in range(14):
                nc.tensor.matmul(
                    wps[:, 0:BCH], wz[:, BCH:BCH + 128], wz[:, 0:BCH],
                    start=True, stop=True, skip_group_check=True,
                )
            nc.vector.tensor_copy(wact[:], wps[:, 0:1])

            # ---------- emit helpers ----------
            def alloc_h1(j):
                # kk in {0,1}: [p, r, q] = h1'[(2kk+r)*128 + p, q]  (fp8)
                return [
                    h1p.tile([128, 2, B], FP8, tag="h1", name=f"h1_{j}_{kk}")
                    for kk in range(2)
                ]

            def emit_l1_block(j, m, w1, h1t):
                """Layer-1 h-chunk m of step j: h1' = relu(8(X@W1) + 8 b1)."""
                bias = B1_sb[:, j * NM + m: j * NM + m + 1]
                for half in range(2):
                    ps = pp.tile([128, BH], F32, tag="ps", name="ps")
                    for bsub in range(2):
                        b = 2 * half + bsub
                        nc.tensor.matmul(
                            ps[:, bsub * BCH:(bsub + 1) * BCH],
                            w1[:, m * 128:(m + 1) * 128],
                            S_sb[:, b * BCH:(b + 1) * BCH],
                            start=True, stop=True,
                        )
                    dst = h1t[m // 2][:, m % 2, half * BH:(half + 1) * BH]
                    if (m + half) % 2 == 0:
                        nc.scalar.activation(dst, ps[:], Relu, bias=bias)
                    else:
                        nc.vector.tensor_scalar(
                            dst, ps[:], bias, 0.0, op0=add, op1=amax,
                        )

            def emit_l2_block(j, m2, w2, h1t, h2):
                """Layer-2 out-chunk m2 (fp8 DoubleRow, K=256/mm):
                h2' = relu(z' + 128 b2), z' accumulated over kk."""
                bias = B2_sb[:, j * NM + m2: j * NM + m2 + 1]
                h2m = h2p.tile([128, B], BF, tag="h2", name=f"h2_{j}_{m2}")
                psh = [
                    pp.tile([128, BH], F32, tag="ps", name=f"psh_{j}_{m2}_{h}")
                    for h in range(2)
                ]
                for kk in range(2):  # kk outer: one weight load per (m2, kk)
                    lhsT = w2[:, 2 * kk:2 * kk + 2, m2 * 128:(m2 + 1) * 128]
                    for half in range(2):
                        for bsub in range(2):
                            b = 2 * half + bsub
                            nc.tensor.matmul(
                                psh[half][:, bsub * BCH:(bsub + 1) * BCH],
                                lhsT,
                                h1t[kk][:, :, b * BCH:(b + 1) * BCH],
                                start=(kk == 0), stop=(kk == 1),
                                perf_mode=DR,
                            )
                for half in range(2):
                    dst = h2m[:, half * BH:(half + 1) * BH]
                    if (m2 + half) % 2 == 0:
                        nc.scalar.activation(dst, psh[half][:], Relu, bias=bias)
                    else:
                        nc.vector.tensor_scalar(
                            dst, psh[half][:], bias, 0.0, op0=add, op1=amax,
                        )
                h2[m2] = h2m

            def emit_l3(j, k, h2):
                """D1[32b+j, :] += w3d_j^T @ h2'[k-chunk][b] (column groups)."""
                c0 = (j * NM + k) * NSTEP
                lhsT = W3D_sb[:, c0: c0 + NSTEP]
                for b in range(NB):
                    nc.tensor.matmul(
                        D1[32 * b: 32 * b + NSTEP, :],
                        lhsT,
                        h2[k][:, b * BCH:(b + 1) * BCH],
                        start=(j == 0 and k == 0),
                        stop=(j == NSTEP - 1 and k == NM - 1),
                        skip_group_check=True,
                        tile_position=(0, 32 * b),
                    )

            # ---------- main pipeline ----------
            # prologue: layer 1 of step 0
            h1_cur = alloc_h1(0)
            for m in range(NM):
                emit_l1_block(0, m, w1_first, h1_cur)

            w2 = w2_first
            prev_h2 = None
            for j in range(NSTEP):
                if j + 1 < NSTEP:
                    w1n = w1p.tile([K2N, H], BF, tag="w1", name=f"w1_{j+1}")
                    nc.sync.dma_start(out=w1n[:], in_=W1_d[j + 1])
                    w2n = w2p.tile([128, NM, H], FP8, tag="w2", name=f"w2_{j+1}")
                    nc.sync.dma_start(
                        out=w2n[:],
                        in_=W2_d[j + 1].rearrange("p (k h) -> p k h", k=NM),
                    )
                    h1_next = alloc_h1(j + 1)
                else:
                    w1n = w2n = h1_next = None

                h2 = {}
                for m2 in range(NM):
                    emit_l2_block(j, m2, w2, h1_cur, h2)
                    if h1_next is not None:
                        emit_l1_block(j + 1, m2, w1n, h1_next)
                    if m2 == 0 and prev_h2 is not None:
                        emit_l3(j - 1, NM - 1, prev_h2)  # deferred last chunk

                for k in range(NM - 1):
                    emit_l3(j, k, h2)

                prev_h2 = h2
                h1_cur = h1_next
                w2 = w2n
            emit_l3(NSTEP - 1, NM - 1, prev_h2)

            # ---- tail: -logp = ln(1 + exp(-sigma*(d + b3d))); two column
            # halves pipelined across DVE (stt, +1) and ACT (exp, ln)
            for ch in range(2):
                sl = slice(ch * (BCH // 2), (ch + 1) * (BCH // 2))
                tt = tailp.tile([128, BCH // 2], F32, tag="tt", name=f"tt{ch}")
                nc.vector.scalar_tensor_tensor(
                    tt[:], D1[:, sl], B3D_sb[:, 0:1], SIG_sb[:, sl],
                    op0=add, op1=mult,
                )
                ex = tailp.tile([128, BCH // 2], F32, tag="ex", name=f"ex{ch}")
                nc.scalar.activation(ex[:], tt[:], Exp, scale=-1.0)
                e1 = tailp.tile([128, BCH // 2], F32, tag="e1", name=f"e1{ch}")
                nc.vector.tensor_scalar(e1[:], ex[:], 1.0, None, op0=add)
                lp = tailp.tile([128, BCH // 2], F32, tag="lp", name=f"lp{ch}")
                nc.scalar.activation(lp[:], e1[:], Ln)
                nc.sync.dma_start(out=OUT_d[:, sl], in_=lp[:])

    return _legalize_waits(_thin_sem_incs(_elide_redundant_ldweights(nc)))


_NC_CACHE = None


def _get_graph():
    global _NC_CACHE
    if _NC_CACHE is None:
        _NC_CACHE = build_graph()
    return _NC_CACHE


def _prep_inputs(samples, W1, b1, W2, b2, W3, b3):
    samples = np.asarray(samples, np.float32)
    W1 = np.asarray(W1, np.float32)
    b1 = np.asarray(b1, np.float32)
    W2 = np.asarray(W2, np.float32)
    b2 = np.asarray(b2, np.float32)
    W3 = np.asarray(W3, np.float32)
    b3 = np.asarray(b3, np.float32)

    # S[2j+s, b] = samples[j, b, s]
    S = samples.transpose(0, 2, 1).reshape(K2N, B).astype(NPBF)
    # mask padded rows (row k of W1[i] is dead unless k < 2i); fold in SH1
    row = np.arange(K2N)[None, :, None]
    step = np.arange(N)[:, None, None]
    W1m = np.where(row < 2 * step, W1 * SH1, 0.0).astype(NPBF)
    w3d = ((W3[:, :, 0] - W3[:, :, 1]) / SZ2).astype(np.float32)  # (N, H)
    b3d = (b3[:, 0] - b3[:, 1]).astype(np.float32)                # (N,)
    sig = (samples[:, :, 0] - samples[:, :, 1]).astype(np.float32)

    in_maps = []
    for c in range(NCORES):
        steps = c + NCORES * np.arange(NSTEP)
        W2c = (
            (W2[steps] * SW2)
            .reshape(NSTEP, NM, 128, H)
            .transpose(0, 2, 1, 3)
            .reshape(NSTEP, 128, NM * H)
            .astype(NPF8)
        )
        B1c = (
            (b1[steps] * SH1).reshape(NSTEP, NM, 128).transpose(2, 0, 1)
            .reshape(128, NSTEP * NM).astype(np.float32)
        )
        B2c = (
            (b2[steps] * SZ2).reshape(NSTEP, NM, 128).transpose(2, 0, 1)
            .reshape(128, NSTEP * NM).astype(np.float32)
        )
        # W3D[p, ((j*NM + k)*NSTEP) + jj] = w3d[steps[j], k*128+p] if jj == j
        W3Dc = np.zeros((128, NSTEP, NM, NSTEP), np.float32)
        for j in range(NSTEP):
            W3Dc[:, j, :, j] = w3d[steps[j]].reshape(NM, 128).T
        W3Dc = W3Dc.reshape(128, NSTEP * NM * NSTEP).astype(NPBF)

        # row 32*b + j layouts for the tail
        SIGc = np.zeros((128, BCH), np.float32)
        SIGc.reshape(NB, 32, BCH)[:, :NSTEP, :] = (
            sig[steps].reshape(NSTEP, NB, BCH).transpose(1, 0, 2)
        )
        B3Dc = np.zeros((128, 1), np.float32)
        B3Dc.reshape(NB, 32)[:, :NSTEP] = b3d[steps][None, :]

        in_maps.append({
            "S": S,
            "W1": np.ascontiguousarray(W1m[steps]),
            "W2": W2c,
            "B1": B1c,
            "B2": B2c,
            "W3D": W3Dc,
            "B3D": B3Dc,
            "SIG": SIGc,
        })
    return in_maps


def kernel(samples, W1, b1, W2, b2, W3, b3):
    global LAST_RESULT
    nc = _get_graph()
    in_maps = _prep_inputs(samples, W1, b1, W2, b2, W3, b3)
    res = run_bass_kernel_spmd(
        nc, in_maps, core_ids=list(range(NCORES)), trace=TRACE,
    )
    LAST_RESULT = res
    # out rows 32*b + j hold -logp for (step j, batch chunk b)
    acc = np.zeros(B, np.float64)
    for c in range(NCORES):
        o = np.asarray(res.results[c]["out"], np.float64)  # [128, 512]
        per_step = o.reshape(NB, 32, BCH)[:, :NSTEP, :]    # (b, j, q)
        acc += per_step.transpose(1, 0, 2).reshape(NSTEP, B).sum(axis=0)
    return (-acc).astype(np.float32).reshape(1, B)
